# revision 64
# baseline (speedup 1.0000x reference)
"""MoE (top-2 of 8 experts, SwiGLU FFN) on 8 Trainium2 NeuronCores.

Strategy (expert-parallel, per the sharding hint):
 - Host: router matmul (f64) + top-2 + softmax gates; dispatch tokens to
   their experts (gather), pad each expert's token list to a uniform
   capacity C.  One expert per core.
 - Device (per core): dense SwiGLU FFN over its C gathered tokens in a
   feature-major (transposed) layout so the per-feature biases are
   per-partition scalars:
       hT = W1[e] @ xgT          (PE, fp16 x fp16 -> fp32 PSUM)
       aT = (h1T+b1a) * silu(h2T+b1b)   (ACT + DVE)
       yT = W2[e] @ aT + b2      (PE, ACT)
 - Host: gather back, apply gate weights, scatter-add into the output.

Shapes are hardcoded for the problem: x [2,2048,1024], E=8, K=2, D=1024,
F=2048.
"""

import os

import numpy as np

import concourse.bacc as bacc
import concourse.tile as tile
from concourse import mybir
from concourse.bass_utils import run_bass_kernel_spmd

B, S, D = 2, 2048, 1024
T = B * S
E = 8
K = 2
F = 2048
TWOF = 2 * F
KT_D = D // 128    # 8 contraction tiles for matmul 1
KT_F = F // 128    # 16 contraction tiles for matmul 2
NF1 = TWOF // 128  # 32 output feature chunks of matmul 1
NF2 = D // 128     # 8 output feature chunks of matmul 2
NT = 512           # token tile (matmul moving free dim)
# w1 chunk storage order (queue-serving): gpsimd slots 0-8, sync 9-15
W1_ORDER = [0, 1, 3, 5, 7, 9, 11, 13, 15, 2, 4, 6, 8, 10, 12, 14]
W1_SLOT = {i: s for s, i in enumerate(W1_ORDER)}

_NC_CACHE = {}
_W_CACHE = {}


def _token_tiles(C):
    """First tile 384: small enough that its xg DMA (the head's serial
    critical path, ~2KB/token) lands early, large enough that per-chunk
    weight consumption (16 matmuls/chunk) stays under the w1 DMA supply
    rate.  Middle tiles 512; last tile in [240, 512] (>=240 keeps the
    128-cycle LDWEIGHTS hidden under the previous matmul), small-ish so
    the post-last-matmul drain (ACT + output DMA) is short."""
    szs = [min(384, C)]
    rem = C - szs[0]
    if 0 < rem < 240:
        take = 240 - rem
        szs[0] -= take
        rem += take
    while rem > 512 + 240:
        szs.append(512)
        rem -= 512
    if rem > 0:
        if rem <= 512:
            szs.append(rem)
        else:
            szs.append(rem - 272)
            szs.append(272)
    tiles = []
    off = 0
    for sz in szs:
        tiles.append((off, sz))
        off += sz
    return tiles


def _build(C):
    """Build the per-core Bass program for capacity C tokens."""
    nc = bacc.Bacc(None, target_bir_lowering=False)
    f16, f32 = mybir.dt.float16, mybir.dt.float32

    # ALL large inputs are host-packed per-partition-contiguous: the DMA
    # elem/packet size equals the min contiguous run of src/dst, and queues
    # moving small (512B) packets get starved ~6:1 by queues moving 4KB
    # packets, so every stream must present >=4KB runs per partition.
    #
    # xgp[p, 8*n0 + k*nsz + c] = x_gathered[token n0+c, k*128+p] for token
    # tile (n0, nsz).
    xgp = nc.dram_tensor("xgp", [128, KT_D * C], f16, kind="ExternalInput")
    # w1q[p, s, k*256+c]: SwiGLU pair chunk (slot s = W1_SLOT[i]) —
    # c<128 -> W1T[k*128+p, i*128+c] (h1), c>=128 -> W1T[k*128+p,
    # F+i*128+(c-128)] (h2).  Chunks are stored in queue-serving order
    # (gpsimd slots 0-8, sync slots 9-15) so each queue's stream can be
    # batched into few DMAs (fewer DMA semaphores -> shorter teardown).
    w1q = nc.dram_tensor("w1q", [128, KT_F, KT_D * 256], f16,
                         kind="ExternalInput")
    # w2q[p, kf, d] = W2T[kf*128+p, d]
    w2q = nc.dram_tensor("w2q", [128, KT_F, D], f16, kind="ExternalInput")
    # bc[:, :NF1] = b1 chunk columns, bc[:, NF1:] = b2 chunk columns
    bc = nc.dram_tensor("bc", [128, NF1 + NF2], f32, kind="ExternalInput")
    ytT = nc.dram_tensor("ytT", [D, C], f32, kind="ExternalOutput")

    Silu = mybir.ActivationFunctionType.Silu
    Ident = mybir.ActivationFunctionType.Identity

    tiles = _token_tiles(C)

    with tile.TileContext(nc) as tc:
        with (
            tc.tile_pool(name="wpool", bufs=1) as wpool,
            tc.tile_pool(name="apool", bufs=2) as apool,
            tc.tile_pool(name="tpool", bufs=4) as tpool,
            tc.tile_pool(name="opool", bufs=4) as opool,
            tc.tile_pool(name="psA", bufs=3, space="PSUM") as psA,
            tc.tile_pool(name="psB", bufs=2, space="PSUM") as psB,
        ):
            # Resident weights / biases / gathered activations.  xg_sb is
            # flat per-tile-block (same layout as xgp) so each tile's DMA is
            # a plain contiguous 2D copy -> 4KB packets, which win a fair
            # share of DMA-engine arbitration against the 4KB w1 stream.
            w1_sb = wpool.tile([128, KT_F, KT_D * 256], f16)
            w2_sb = wpool.tile([128, KT_F, D], f16)
            xg_sb = wpool.tile([128, KT_D * C], f16)
            bc_sb = wpool.tile([128, NF1 + NF2], f32)

            def xg_mv(n0, nsz, k):
                # moving operand [128, nsz] for contraction block k of the
                # token tile at (n0, nsz)
                base = KT_D * n0 + k * nsz
                return xg_sb[:, base:base + nsz]

            # Warm-up matmuls on a zeroed tile: keeps the PE busy during the
            # initial DMA wait so HAM unthrottles (1.2 -> 2.4 GHz) before
            # the first real matmul.  Each warmup is ~107ns (LDWEIGHTS-
            # bound); preamble ends ~7us, first tile's data lands ~13.5us
            # -> 64 covers ~6.8us.
            warm_sb = wpool.tile([128, 128], f16)
            nc.vector.memset(warm_sb, 0.0)
            warm_ps = psB.tile([128, NT], f32, tag="psb")
            for _ in range(64):
                nc.tensor.matmul(warm_ps[:, :128], warm_sb, warm_sb,
                                 start=True, stop=True)

            ytr = ytT.rearrange("(j p) c -> p j c", p=128)
            # DMA scheduling.  Weights ride TWO queues (gpsimd + sync): a
            # single queue sustains only ~150-250 GB/s while the fabric
            # aggregates ~300-450, so alternating w1 chunks across both
            # roughly doubles weight-delivery rate.  The sync queue leads
            # with xg tile 0 and most of w1 chunk 0's tail (sync's queue
            # starts flowing ~1.8us before gpsimd's); later xg tiles ride
            # at the END of sync's weight stream — queue FIFO keeps them
            # out of the head and they still land long before tile 1 needs
            # them.  w2 halves ride at both queue tails (first needed at
            # mm2 of tile 0, which tile0=384 delays past their arrival).
            # The scalar engine only issues the tiny bias DMA, staying
            # free for ACT_TABLE_LOAD before the first silu.
            # xg tile 0 and w1 chunk 0 each ship as 3 k-block-aligned
            # pieces: the first PSUM group's matmul k only reads block k,
            # so finer DMA granularity lets matmuls start as soon as the
            # leading blocks land instead of waiting for the whole
            # transfer's completion semaphore.
            n0, nsz = tiles[0]
            for lo, hi in ((0, 3), (3, 6), (6, 8)):
                nc.sync.dma_start(out=xg_sb[:, lo * nsz:hi * nsz],
                                  in_=xgp[:, lo * nsz:hi * nsz])
            for lo, hi in ((0, 768), (768, 1536), (1536, 2048)):
                nc.gpsimd.dma_start(out=w1_sb[:, 0, lo:hi],
                                    in_=w1q[:, 0, lo:hi])
            nc.scalar.dma_start(out=bc_sb, in_=bc[:, :])
            # gpsimd: slots 1-8 (logical c1,c3..c15); sync: slots 9-15
            # (c2,c4..c14) behind xg tile 0.  Per-chunk DMAs: batching
            # chunks (even late ones) delays completion semaphores and
            # perturbs queue dynamics enough to cost more than the saved
            # teardown semaphores.
            for s in range(1, 9):
                nc.gpsimd.dma_start(out=w1_sb[:, s, :], in_=w1q[:, s, :])
            for s in range(9, 16):
                nc.sync.dma_start(out=w1_sb[:, s, :], in_=w1q[:, s, :])
            nc.gpsimd.dma_start(out=w2_sb[:, 0:8, :], in_=w2q[:, 0:8, :])
            nc.sync.dma_start(out=w2_sb[:, 8:16, :], in_=w2q[:, 8:16, :])
            for m0, msz in tiles[1:]:
                nc.sync.dma_start(
                    out=xg_sb[:, KT_D * m0:KT_D * (m0 + msz)],
                    in_=xgp[:, KT_D * m0:KT_D * (m0 + msz)])

            for ti, (n0, nsz) in enumerate(tiles):
                a_t = apool.tile([128, KT_F, NT], f16, tag="a")
                # ---- matmul 1 + SwiGLU: aT = (h1+b1a) * silu(h2+b1b)
                for i in range(KT_F):
                    ps1 = psA.tile([128, NT], f32, tag="ps1")
                    ps2 = psA.tile([128, NT], f32, tag="ps2")
                    si = W1_SLOT[i]
                    for k in range(KT_D):
                        nc.tensor.matmul(
                            ps1[:, :nsz],
                            w1_sb[:, si, k * 256:k * 256 + 128],
                            xg_mv(n0, nsz, k),
                            start=(k == 0),
                            stop=(k == KT_D - 1),
                        )
                    for k in range(KT_D):
                        nc.tensor.matmul(
                            ps2[:, :nsz],
                            w1_sb[:, si, k * 256 + 128:k * 256 + 256],
                            xg_mv(n0, nsz, k),
                            start=(k == 0),
                            stop=(k == KT_D - 1),
                        )
                    s_t = tpool.tile([128, NT], f32, tag="s")
                    nc.scalar.activation(
                        s_t[:, :nsz], ps2[:, :nsz], Silu,
                        bias=bc_sb[:, KT_F + i:KT_F + i + 1],
                    )
                    h_t = tpool.tile([128, NT], f32, tag="h")
                    nc.vector.tensor_scalar_add(
                        h_t[:, :nsz], ps1[:, :nsz], bc_sb[:, i:i + 1]
                    )
                    nc.vector.tensor_mul(
                        a_t[:, i, :nsz], h_t[:, :nsz], s_t[:, :nsz]
                    )

                # ---- matmul 2: yT = W2 @ aT + b2
                # j0-j3 outputs batched into one DMA; j4-j7 individual so
                # the post-last-matmul drain transfer stays small.
                o4_t = opool.tile([128, 4, NT], f32, tag="o4")
                for j in range(NF2):
                    ps = psB.tile([128, NT], f32, tag="psb")
                    for kf in range(KT_F):
                        nc.tensor.matmul(
                            ps[:, :nsz],
                            w2_sb[:, kf, j * 128:(j + 1) * 128],
                            a_t[:, kf, :nsz],
                            start=(kf == 0),
                            stop=(kf == KT_F - 1),
                        )
                    if j < 4:
                        nc.scalar.activation(
                            o4_t[:, j, :nsz], ps[:, :nsz], Ident,
                            bias=bc_sb[:, NF1 + j:NF1 + j + 1],
                        )
                        if j == 3:
                            nc.sync.dma_start(
                                out=ytr[:, 0:4, n0:n0 + nsz],
                                in_=o4_t[:, :, :nsz],
                            )
                    else:
                        o_t = opool.tile([128, NT], f32, tag="o")
                        nc.scalar.activation(
                            o_t[:, :nsz], ps[:, :nsz], Ident,
                            bias=bc_sb[:, NF1 + j:NF1 + j + 1],
                        )
                        nc.sync.dma_start(
                            out=ytr[:, j, n0:n0 + nsz],
                            in_=o_t[:, :nsz],
                        )
    nc.compile()
    return nc


def _get_nc(C):
    nc = _NC_CACHE.get(C)
    if nc is None:
        nc = _build(C)
        _NC_CACHE[C] = nc
    return nc


def _weights16(W1, W2):
    key = (W1.shape, W2.shape, W1.dtype.str, bytes(np.asarray(W1[0, 0, :8]).data),
           bytes(np.asarray(W2[0, 0, :8]).data))
    hit = _W_CACHE.get("w")
    if hit is not None and hit[0] == key:
        return hit[1], hit[2]
    # W1Q[e, p, i, k*256+c]: pair-packed W1 chunks (h1 half then h2 half of
    # SwiGLU chunk i), per-partition-contiguous (4KB/partition per chunk).
    W1T = np.transpose(W1, (0, 2, 1)).astype(np.float16)  # [E, D, 2F]
    W1r = W1T.reshape(E, KT_D, 128, 2, KT_F, 128)  # [e, k, p, half, i, c]
    W1Q = np.transpose(W1r, (0, 2, 4, 1, 3, 5)).reshape(
        E, 128, KT_F, KT_D * 256)
    W1Q = np.ascontiguousarray(W1Q[:, :, W1_ORDER, :])
    W2T = np.transpose(W2, (0, 2, 1)).astype(np.float16)  # [E, F, D]
    # W2Q[e, p, kf, d] = W2T[e, kf*128+p, d]
    W2Q = np.ascontiguousarray(
        np.transpose(W2T.reshape(E, KT_F, 128, D), (0, 2, 1, 3)))
    _W_CACHE["w"] = (key, W1Q, W2Q)
    return W1Q, W2Q


def kernel(x, Wr, temp, W1, b1, W2, b2):
    x = np.asarray(x)
    xf = np.ascontiguousarray(x.reshape(T, D), dtype=np.float32)

    # ---- host router (f64 for a stable top-k ordering)
    logits = xf.astype(np.float64) @ np.asarray(Wr).astype(np.float64).T
    logits /= np.float64(np.asarray(temp).reshape(-1)[0])
    top_idx = np.argsort(-logits, axis=1, kind="stable")[:, :K]  # [T, K]
    top_v = np.take_along_axis(logits, top_idx, axis=1)
    top_v -= top_v.max(axis=1, keepdims=True)
    ex = np.exp(top_v)
    gates = (ex / ex.sum(axis=1, keepdims=True)).astype(np.float32)  # [T, K]

    # ---- dispatch: per-expert token lists
    idx_e = []
    gate_e = []
    for e in range(E):
        rows, slot = np.where(top_idx == e)
        idx_e.append(rows)
        gate_e.append(gates[rows, slot])
    counts = np.array([len(r) for r in idx_e])
    C = max(256, int(-(-counts.max() // 16) * 16))

    nc = _get_nc(C)

    xf16 = xf.astype(np.float16)
    W1Q, W2Q = _weights16(np.asarray(W1), np.asarray(W2))
    b1a = np.asarray(b1, dtype=np.float32)  # [E, 2F]
    b2a = np.asarray(b2, dtype=np.float32)  # [E, D]

    tiles = _token_tiles(C)
    in_maps = []
    for e in range(E):
        xg = np.zeros((C, D), np.float16)
        xg[:counts[e]] = xf16[idx_e[e]]
        # pack per token tile: xgp[p, 8*n0 + k*nsz + c] = xg[n0+c, k*128+p]
        xgk = xg.reshape(C, KT_D, 128)  # [tok, k, p]
        xgp = np.empty((128, KT_D * C), np.float16)
        for n0, nsz in tiles:
            blk = np.transpose(xgk[n0:n0 + nsz], (2, 1, 0))  # [p, k, tok]
            xgp[:, KT_D * n0:KT_D * (n0 + nsz)] = blk.reshape(128, -1)
        bc = np.concatenate(
            [b1a[e].reshape(NF1, 128).T, b2a[e].reshape(NF2, 128).T], axis=1)
        in_maps.append({
            "xgp": xgp,
            "w1q": W1Q[e],
            "w2q": W2Q[e],
            "bc": np.ascontiguousarray(bc),
        })

    kwargs = {}
    if os.environ.get("KERNEL_TRACE"):
        kwargs = {"trace": True}
    try:
        res = run_bass_kernel_spmd(nc, in_maps, core_ids=list(range(E)), **kwargs)
    except ModuleNotFoundError:
        # trace path needs antenv.axon_hooks, absent on some images
        os.environ["BASS_NEVER_TRACE"] = "1"
        res = run_bass_kernel_spmd(nc, in_maps, core_ids=list(range(E)))
    global LAST_RESULT
    LAST_RESULT = res

    out = np.zeros((T, D), np.float32)
    for e in range(E):
        cnt = counts[e]
        if cnt == 0:
            continue
        y = res.results[e]["ytT"][:, :cnt].T  # [cnt, D]
        # top-2 expert choices are distinct, so rows are unique per expert
        out[idx_e[e]] += gate_e[e][:, None] * y
    return out.reshape(B, S, D)


LAST_RESULT = None



# revision 65
# speedup vs baseline: 1.1911x; 1.1911x over previous
"""MoE (top-2 of 8 experts, SwiGLU FFN) on 8 Trainium2 NeuronCores.

Strategy (expert-parallel, per the sharding hint):
 - Host: router matmul (f64) + top-2 + softmax gates; dispatch tokens to
   their experts (gather), pad each expert's token list to a uniform
   capacity C.  One expert per core.
 - Device (per core): dense SwiGLU FFN over its C gathered tokens in a
   feature-major (transposed) layout so the per-feature biases are
   per-partition scalars:
       hT = W1[e] @ xgT          (PE, fp16 x fp16 -> fp32 PSUM)
       aT = (h1T+b1a) * silu(h2T+b1b)   (ACT + DVE)
       yT = W2[e] @ aT + b2      (PE, ACT)
 - Host: gather back, apply gate weights, scatter-add into the output.

Shapes are hardcoded for the problem: x [2,2048,1024], E=8, K=2, D=1024,
F=2048.
"""

import os

import numpy as np

import concourse.bacc as bacc
import concourse.tile as tile
from concourse import mybir
from concourse.bass_utils import run_bass_kernel_spmd

B, S, D = 2, 2048, 1024
T = B * S
E = 8
K = 2
F = 2048
TWOF = 2 * F
KT_D = D // 128    # 8 contraction tiles for matmul 1
KT_F = F // 128    # 16 contraction tiles for matmul 2
NF1 = TWOF // 128  # 32 output feature chunks of matmul 1
NF2 = D // 128     # 8 output feature chunks of matmul 2
NT = 512           # token tile (matmul moving free dim)
# w1 chunk storage order (queue-serving): gpsimd slots 0-8, sync 9-15
W1_ORDER = [0, 1, 3, 5, 7, 9, 11, 13, 15, 2, 4, 6, 8, 10, 12, 14]
W1_SLOT = {i: s for s, i in enumerate(W1_ORDER)}

_NC_CACHE = {}
_W_CACHE = {}


def _token_tiles(C):
    """First tile 384: small enough that its xg DMA (the head's serial
    critical path, ~2KB/token) lands early, large enough that per-chunk
    weight consumption (16 matmuls/chunk) stays under the w1 DMA supply
    rate.  Middle tiles 512; last tile in [240, 512] (>=240 keeps the
    128-cycle LDWEIGHTS hidden under the previous matmul), small-ish so
    the post-last-matmul drain (ACT + output DMA) is short."""
    szs = [min(384, C)]
    rem = C - szs[0]
    if 0 < rem < 240:
        take = 240 - rem
        szs[0] -= take
        rem += take
    while rem > 512 + 240:
        szs.append(512)
        rem -= 512
    if rem > 0:
        if rem <= 512:
            szs.append(rem)
        else:
            szs.append(rem - 272)
            szs.append(272)
    tiles = []
    off = 0
    for sz in szs:
        tiles.append((off, sz))
        off += sz
    return tiles


def _build(C):
    """Build the per-core Bass program for capacity C tokens."""
    nc = bacc.Bacc(None, target_bir_lowering=False)
    f16, f32 = mybir.dt.float16, mybir.dt.float32

    # ALL large inputs are host-packed per-partition-contiguous: the DMA
    # elem/packet size equals the min contiguous run of src/dst, and queues
    # moving small (512B) packets get starved ~6:1 by queues moving 4KB
    # packets, so every stream must present >=4KB runs per partition.
    #
    # xgp[p, 8*n0 + k*nsz + c] = x_gathered[token n0+c, k*128+p] for token
    # tile (n0, nsz).
    xgp = nc.dram_tensor("xgp", [128, KT_D * C], f16, kind="ExternalInput")
    # w1q[p, s, k*256+c]: SwiGLU pair chunk (slot s = W1_SLOT[i]) —
    # c<128 -> W1T[k*128+p, i*128+c] (h1), c>=128 -> W1T[k*128+p,
    # F+i*128+(c-128)] (h2).  Chunks are stored in queue-serving order
    # (gpsimd slots 0-8, sync slots 9-15) so each queue's stream can be
    # batched into few DMAs (fewer DMA semaphores -> shorter teardown).
    w1q = nc.dram_tensor("w1q", [128, KT_F, KT_D * 256], f16,
                         kind="ExternalInput")
    # w2q[p, kf, d] = W2T[kf*128+p, d]
    w2q = nc.dram_tensor("w2q", [128, KT_F, D], f16, kind="ExternalInput")
    # bc[:, :NF1] = b1 chunk columns, bc[:, NF1:] = b2 chunk columns
    bc = nc.dram_tensor("bc", [128, NF1 + NF2], f32, kind="ExternalInput")
    ytT = nc.dram_tensor("ytT", [D, C], f32, kind="ExternalOutput")

    Silu = mybir.ActivationFunctionType.Silu
    Ident = mybir.ActivationFunctionType.Identity

    tiles = _token_tiles(C)

    with tile.TileContext(nc) as tc:
        with (
            tc.tile_pool(name="wpool", bufs=1) as wpool,
            tc.tile_pool(name="apool", bufs=2) as apool,
            tc.tile_pool(name="tpool", bufs=4) as tpool,
            tc.tile_pool(name="opool", bufs=4) as opool,
            tc.tile_pool(name="psA", bufs=3, space="PSUM") as psA,
            tc.tile_pool(name="psB", bufs=2, space="PSUM") as psB,
        ):
            # Resident weights / biases / gathered activations.  xg_sb is
            # flat per-tile-block (same layout as xgp) so each tile's DMA is
            # a plain contiguous 2D copy -> 4KB packets, which win a fair
            # share of DMA-engine arbitration against the 4KB w1 stream.
            w1_sb = wpool.tile([128, KT_F, KT_D * 256], f16)
            w2_sb = wpool.tile([128, KT_F, D], f16)
            xg_sb = wpool.tile([128, KT_D * C], f16)
            bc_sb = wpool.tile([128, NF1 + NF2], f32)

            def xg_mv(n0, nsz, k):
                # moving operand [128, nsz] for contraction block k of the
                # token tile at (n0, nsz)
                base = KT_D * n0 + k * nsz
                return xg_sb[:, base:base + nsz]

            # Warm-up matmuls on a zeroed tile: keeps the PE busy during the
            # initial DMA wait so HAM unthrottles (1.2 -> 2.4 GHz) before
            # the first real matmul.  Each warmup is ~107ns (LDWEIGHTS-
            # bound); preamble ends ~7us, first tile's data lands ~13.5us
            # -> 64 covers ~6.8us.
            warm_sb = wpool.tile([128, 128], f16)
            nc.vector.memset(warm_sb, 0.0)
            warm_ps = psB.tile([128, NT], f32, tag="psb")
            for _ in range(64):
                nc.tensor.matmul(warm_ps[:, :128], warm_sb, warm_sb,
                                 start=True, stop=True)

            ytr = ytT.rearrange("(j p) c -> p j c", p=128)
            # DMA scheduling.  Weights ride TWO queues (gpsimd + sync): a
            # single queue sustains only ~150-250 GB/s while the fabric
            # aggregates ~300-450, so alternating w1 chunks across both
            # roughly doubles weight-delivery rate.  The sync queue leads
            # with xg tile 0 and most of w1 chunk 0's tail (sync's queue
            # starts flowing ~1.8us before gpsimd's); later xg tiles ride
            # at the END of sync's weight stream — queue FIFO keeps them
            # out of the head and they still land long before tile 1 needs
            # them.  w2 halves ride at both queue tails (first needed at
            # mm2 of tile 0, which tile0=384 delays past their arrival).
            # The scalar engine only issues the tiny bias DMA, staying
            # free for ACT_TABLE_LOAD before the first silu.
            # NOTE: finer-grained head DMAs (k-block splits of xg tile 0 /
            # w1 chunk 0) and some tilings ([448,352,272]) reproducibly
            # trigger a ~1.97GHz PE clock state (+19% on every matmul,
            # HAM still reporting k=8/8) — keep the head as two coarse
            # transfers plus a small sync-side tail of chunk 0.
            n0, nsz = tiles[0]
            nc.sync.dma_start(out=xg_sb[:, :KT_D * nsz],
                              in_=xgp[:, :KT_D * nsz])
            nc.gpsimd.dma_start(out=w1_sb[:, 0, 0:1792], in_=w1q[:, 0, 0:1792])
            nc.scalar.dma_start(out=bc_sb, in_=bc[:, :])
            nc.sync.dma_start(out=w1_sb[:, 0, 1792:2048],
                              in_=w1q[:, 0, 1792:2048])
            # gpsimd: slots 1-8 (logical c1,c3..c15); sync: slots 9-15
            # (c2,c4..c14) behind xg tile 0.  Per-chunk DMAs: batching
            # chunks (even late ones) delays completion semaphores and
            # perturbs queue dynamics enough to cost more than the saved
            # teardown semaphores.
            for s in range(1, 9):
                nc.gpsimd.dma_start(out=w1_sb[:, s, :], in_=w1q[:, s, :])
            for s in range(9, 16):
                nc.sync.dma_start(out=w1_sb[:, s, :], in_=w1q[:, s, :])
            nc.gpsimd.dma_start(out=w2_sb[:, 0:8, :], in_=w2q[:, 0:8, :])
            nc.sync.dma_start(out=w2_sb[:, 8:16, :], in_=w2q[:, 8:16, :])
            for m0, msz in tiles[1:]:
                nc.sync.dma_start(
                    out=xg_sb[:, KT_D * m0:KT_D * (m0 + msz)],
                    in_=xgp[:, KT_D * m0:KT_D * (m0 + msz)])

            for ti, (n0, nsz) in enumerate(tiles):
                a_t = apool.tile([128, KT_F, NT], f16, tag="a")
                # ---- matmul 1 + SwiGLU: aT = (h1+b1a) * silu(h2+b1b)
                for i in range(KT_F):
                    ps1 = psA.tile([128, NT], f32, tag="ps1")
                    ps2 = psA.tile([128, NT], f32, tag="ps2")
                    si = W1_SLOT[i]
                    for k in range(KT_D):
                        nc.tensor.matmul(
                            ps1[:, :nsz],
                            w1_sb[:, si, k * 256:k * 256 + 128],
                            xg_mv(n0, nsz, k),
                            start=(k == 0),
                            stop=(k == KT_D - 1),
                        )
                    for k in range(KT_D):
                        nc.tensor.matmul(
                            ps2[:, :nsz],
                            w1_sb[:, si, k * 256 + 128:k * 256 + 256],
                            xg_mv(n0, nsz, k),
                            start=(k == 0),
                            stop=(k == KT_D - 1),
                        )
                    s_t = tpool.tile([128, NT], f32, tag="s")
                    nc.scalar.activation(
                        s_t[:, :nsz], ps2[:, :nsz], Silu,
                        bias=bc_sb[:, KT_F + i:KT_F + i + 1],
                    )
                    h_t = tpool.tile([128, NT], f32, tag="h")
                    nc.vector.tensor_scalar_add(
                        h_t[:, :nsz], ps1[:, :nsz], bc_sb[:, i:i + 1]
                    )
                    nc.vector.tensor_mul(
                        a_t[:, i, :nsz], h_t[:, :nsz], s_t[:, :nsz]
                    )

                # ---- matmul 2: yT = W2 @ aT + b2
                # j0-j3 outputs batched into one DMA; j4-j7 individual so
                # the post-last-matmul drain transfer stays small.
                o4_t = opool.tile([128, 4, NT], f32, tag="o4")
                for j in range(NF2):
                    ps = psB.tile([128, NT], f32, tag="psb")
                    for kf in range(KT_F):
                        nc.tensor.matmul(
                            ps[:, :nsz],
                            w2_sb[:, kf, j * 128:(j + 1) * 128],
                            a_t[:, kf, :nsz],
                            start=(kf == 0),
                            stop=(kf == KT_F - 1),
                        )
                    if j < 4:
                        nc.scalar.activation(
                            o4_t[:, j, :nsz], ps[:, :nsz], Ident,
                            bias=bc_sb[:, NF1 + j:NF1 + j + 1],
                        )
                        if j == 3:
                            nc.sync.dma_start(
                                out=ytr[:, 0:4, n0:n0 + nsz],
                                in_=o4_t[:, :, :nsz],
                            )
                    else:
                        o_t = opool.tile([128, NT], f32, tag="o")
                        nc.scalar.activation(
                            o_t[:, :nsz], ps[:, :nsz], Ident,
                            bias=bc_sb[:, NF1 + j:NF1 + j + 1],
                        )
                        nc.sync.dma_start(
                            out=ytr[:, j, n0:n0 + nsz],
                            in_=o_t[:, :nsz],
                        )
    nc.compile()
    return nc


def _get_nc(C):
    nc = _NC_CACHE.get(C)
    if nc is None:
        nc = _build(C)
        _NC_CACHE[C] = nc
    return nc


def _weights16(W1, W2):
    key = (W1.shape, W2.shape, W1.dtype.str, bytes(np.asarray(W1[0, 0, :8]).data),
           bytes(np.asarray(W2[0, 0, :8]).data))
    hit = _W_CACHE.get("w")
    if hit is not None and hit[0] == key:
        return hit[1], hit[2]
    # W1Q[e, p, i, k*256+c]: pair-packed W1 chunks (h1 half then h2 half of
    # SwiGLU chunk i), per-partition-contiguous (4KB/partition per chunk).
    W1T = np.transpose(W1, (0, 2, 1)).astype(np.float16)  # [E, D, 2F]
    W1r = W1T.reshape(E, KT_D, 128, 2, KT_F, 128)  # [e, k, p, half, i, c]
    W1Q = np.transpose(W1r, (0, 2, 4, 1, 3, 5)).reshape(
        E, 128, KT_F, KT_D * 256)
    W1Q = np.ascontiguousarray(W1Q[:, :, W1_ORDER, :])
    W2T = np.transpose(W2, (0, 2, 1)).astype(np.float16)  # [E, F, D]
    # W2Q[e, p, kf, d] = W2T[e, kf*128+p, d]
    W2Q = np.ascontiguousarray(
        np.transpose(W2T.reshape(E, KT_F, 128, D), (0, 2, 1, 3)))
    _W_CACHE["w"] = (key, W1Q, W2Q)
    return W1Q, W2Q


def kernel(x, Wr, temp, W1, b1, W2, b2):
    x = np.asarray(x)
    xf = np.ascontiguousarray(x.reshape(T, D), dtype=np.float32)

    # ---- host router (f64 for a stable top-k ordering)
    logits = xf.astype(np.float64) @ np.asarray(Wr).astype(np.float64).T
    logits /= np.float64(np.asarray(temp).reshape(-1)[0])
    top_idx = np.argsort(-logits, axis=1, kind="stable")[:, :K]  # [T, K]
    top_v = np.take_along_axis(logits, top_idx, axis=1)
    top_v -= top_v.max(axis=1, keepdims=True)
    ex = np.exp(top_v)
    gates = (ex / ex.sum(axis=1, keepdims=True)).astype(np.float32)  # [T, K]

    # ---- dispatch: per-expert token lists
    idx_e = []
    gate_e = []
    for e in range(E):
        rows, slot = np.where(top_idx == e)
        idx_e.append(rows)
        gate_e.append(gates[rows, slot])
    counts = np.array([len(r) for r in idx_e])
    C = max(256, int(-(-counts.max() // 16) * 16))

    nc = _get_nc(C)

    xf16 = xf.astype(np.float16)
    W1Q, W2Q = _weights16(np.asarray(W1), np.asarray(W2))
    b1a = np.asarray(b1, dtype=np.float32)  # [E, 2F]
    b2a = np.asarray(b2, dtype=np.float32)  # [E, D]

    tiles = _token_tiles(C)
    in_maps = []
    for e in range(E):
        xg = np.zeros((C, D), np.float16)
        xg[:counts[e]] = xf16[idx_e[e]]
        # pack per token tile: xgp[p, 8*n0 + k*nsz + c] = xg[n0+c, k*128+p]
        xgk = xg.reshape(C, KT_D, 128)  # [tok, k, p]
        xgp = np.empty((128, KT_D * C), np.float16)
        for n0, nsz in tiles:
            blk = np.transpose(xgk[n0:n0 + nsz], (2, 1, 0))  # [p, k, tok]
            xgp[:, KT_D * n0:KT_D * (n0 + nsz)] = blk.reshape(128, -1)
        bc = np.concatenate(
            [b1a[e].reshape(NF1, 128).T, b2a[e].reshape(NF2, 128).T], axis=1)
        in_maps.append({
            "xgp": xgp,
            "w1q": W1Q[e],
            "w2q": W2Q[e],
            "bc": np.ascontiguousarray(bc),
        })

    kwargs = {}
    if os.environ.get("KERNEL_TRACE"):
        kwargs = {"trace": True}
    try:
        res = run_bass_kernel_spmd(nc, in_maps, core_ids=list(range(E)), **kwargs)
    except ModuleNotFoundError:
        # trace path needs antenv.axon_hooks, absent on some images
        os.environ["BASS_NEVER_TRACE"] = "1"
        res = run_bass_kernel_spmd(nc, in_maps, core_ids=list(range(E)))
    global LAST_RESULT
    LAST_RESULT = res

    out = np.zeros((T, D), np.float32)
    for e in range(E):
        cnt = counts[e]
        if cnt == 0:
            continue
        y = res.results[e]["ytT"][:, :cnt].T  # [cnt, D]
        # top-2 expert choices are distinct, so rows are unique per expert
        out[idx_e[e]] += gate_e[e][:, None] * y
    return out.reshape(B, S, D)


LAST_RESULT = None

